# revision 10
# baseline (speedup 1.0000x reference)
"""Trainium2 Bass kernel for nn_CausalTrajectoryPrediction.

Per-node stacked MLP over B=16384 rows, N=64 nodes:
  x1[b,i,:] = x[b,:] with entry i zeroed       (mask folded into weights host-side)
  z_i  = relu(W1a'_i @ x) , relu(W2a'_i @ x)   (two branches, packed M=128)
  r_i  = relu(blockdiag(W1b_i, W2b_i) @ z_i)   (K=128, M=64)
  h_i  = relu(W3ab_i @ r_i + w3x_i * x[:,i] + b3a_i)
  out  = relu(w3b_i . h_i + b3b_i)             (final bias+relu on host)

Layout: activations transposed [feature, B]; batch sharded across 8 cores
(BL=2048 each); nodes processed in pairs so every ACT/DVE op uses 128
partitions; matmul groups are subarray-tiled via tile_position for PE
concurrency.  Inputs arrive as prepacked DRAM tensors (host does all
transposes/masking); built on Bacc so multi-semaphore waits are split
into EventSemaphores (walrus allows one wait per Matmult).

Perf notes (measured on 8x trn2; 868us fp32 -> 247us bf16 -> this):
- all matmul operands bf16 (1 PE cycle/row vs 4 for fp32); PSUM stays
  fp32.  fp8 is out: e4m3 on L1 alone already gives 2.7e-2 rel err.
- row-disjoint matmul pairs share one N=512 stream window on the PE.
- x-passthrough AND b3a are folded into ONE K=3 matmul per pair via
  xt4 (host-packed [x_2t, x_2t+1, ones, 0] partition quads) and w3xb
  (lhsT rows = [w3x_i0|0], [0|w3x_i1], [b3a_i0|b3a_i1]).  Pairs t and
  t+16 sit in disjoint 32-row strips of xt4, so their K=3 mms share
  one stream window (x+bias cost halves vs one window per pair, and
  the ACT g-relu loses its bias read).
- super-iteration (t, t+16): h is one [128,1024] PSUM tile; per-pair
  g-relus read halves.  PSUM banks: z 4 + r 1 + h 2 + o 1 = 8 (full).
- L4(prev) is issued before main/g of the current pair: it depends on
  an ACT output (g), so issuing it late avoids head-of-line-blocking
  the in-order PE queue; z relu is one merged [128,1024] DVE op.
- Anti-diagonal tile positions (64,0)/(0,64) crash at runtime (HW
  quadrant bug); GPSIMD cannot read PSUM, so relus stay on ACT/DVE.
"""

import numpy as np
from contextlib import ExitStack

N, H, M, B = 64, 64, 32, 16384
NCORES = 8
BL = B // NCORES            # 2048 batch columns per core
CH = 512                    # chunk width (one PSUM bank of fp32)
NPAIR = N // 2              # 32 node pairs

_cache = {}


def _build_bass(bl, npair):
    import concourse.bass as bass
    import concourse.bacc as bacc
    import concourse.mybir as mybir
    import concourse.tile as tile

    F32 = mybir.dt.float32
    BF16 = mybir.dt.bfloat16
    Relu = mybir.ActivationFunctionType.Relu
    Copy = mybir.ActivationFunctionType.Copy
    nch = bl // CH

    nc = bacc.Bacc()
    xt_d = nc.dram_tensor("xt", [128, bl], BF16, kind="ExternalInput")
    xt4_d = nc.dram_tensor("xt4", [96, bl], BF16, kind="ExternalInput")
    w1_d = nc.dram_tensor("w1", [128, npair * 128], BF16, kind="ExternalInput")
    w2_d = nc.dram_tensor("w2", [128, npair * 128], BF16, kind="ExternalInput")
    w3_d = nc.dram_tensor("w3", [128, npair * 128], BF16, kind="ExternalInput")
    w3xb_d = nc.dram_tensor("w3xb", [128, npair * 128], BF16, kind="ExternalInput")
    w4_d = nc.dram_tensor("w4", [128, npair * 2], BF16, kind="ExternalInput")
    out_d = nc.dram_tensor("opre", [bl, N], F32, kind="ExternalOutput")

    mm = nc.tensor.matmul  # bf16 operands: 1 PE cycle/row (fp32 was 4)

    with tile.TileContext(nc) as tc, ExitStack() as ctx:
        wpool = ctx.enter_context(tc.tile_pool(name="weights", bufs=1))
        apool = ctx.enter_context(tc.tile_pool(name="acts", bufs=2))
        ps_z = ctx.enter_context(tc.tile_pool(name="ps_z", bufs=2, space="PSUM"))
        ps_r = ctx.enter_context(tc.tile_pool(name="ps_r", bufs=1, space="PSUM"))
        ps_h = ctx.enter_context(tc.tile_pool(name="ps_h", bufs=1, space="PSUM"))
        ps_o = ctx.enter_context(tc.tile_pool(name="ps_o", bufs=1, space="PSUM"))

        # Parallel HWDGE loads; xt/w1 first so L1 compute starts ASAP.
        xt_sb = wpool.tile([128, bl], BF16, tag="xt")
        nc.sync.dma_start(xt_sb[:, 0:CH], xt_d[:, 0:CH])
        w1_sb = wpool.tile([128, npair * 128], BF16, tag="w1")
        nc.sync.dma_start(w1_sb[:, 0:512], w1_d[:, 0:512])
        nc.sync.dma_start(xt_sb[:, CH:bl], xt_d[:, CH:bl])
        nc.sync.dma_start(w1_sb[:, 512 : npair * 128], w1_d[:, 512 : npair * 128])
        w2_sb = wpool.tile([128, npair * 128], BF16, tag="w2")
        nc.sync.dma_start(w2_sb[:, 0:512], w2_d[:, 0:512])
        nc.sync.dma_start(w2_sb[:, 512 : npair * 128], w2_d[:, 512 : npair * 128])
        # xt4: matmul APs must start at 32-aligned partitions, so pair t's
        # [x_2t, x_2t+1, ones] rows land at partition 32*(t%4) in batch-slot
        # t//4 (8 slots along the free axis), scattered from a compact
        # [96, bl] DRAM tensor
        xt4_sb = wpool.tile([128, 8 * bl], BF16, tag="xt4")
        for t in range(npair):
            nc.sync.dma_start(
                xt4_sb[32 * (t % 4) : 32 * (t % 4) + 3,
                       (t // 4) * bl : (t // 4) * bl + bl],
                xt4_d[3 * t : 3 * t + 3, :])
        w3_sb = wpool.tile([128, npair * 128], BF16, tag="w3")
        nc.sync.dma_start(w3_sb[:, 0:512], w3_d[:, 0:512])
        nc.sync.dma_start(w3_sb[:, 512 : npair * 128], w3_d[:, 512 : npair * 128])
        w3xb_sb = wpool.tile([128, npair * 128], BF16, tag="w3xb")
        nc.sync.dma_start(w3xb_sb[:], w3xb_d[:])
        w4_sb = wpool.tile([128, npair * 2], BF16, tag="w4")
        nc.sync.dma_start(w4_sb[:], w4_d[:])

        # super-iterations: (chunk c, pair 2k, pair 2k+1) — adjacent pairs
        # sit in adjacent 32-row strips (32*(t%4)), so their K=3 x+bias mms
        # are row-disjoint and share one PE window
        siters = [(c, 2 * k, 2 * k + 1) for c in range(nch)
                  for k in range(npair // 2)]
        obanks = {}

        def emit_L1(c, t):
            # L1: both branches for each node of the pair (K=64, M=128);
            # the two nodes run row-concurrent on the PE (xt duplicated at
            # partitions 64-127); one merged [128,1024] relu on DVE
            xt_c = xt_sb[:, c * CH : (c + 1) * CH]
            w1t = w1_sb[:, t * 128 : (t + 1) * 128]
            z_ps = ps_z.tile([128, 2 * CH], F32, tag="z", name=f"z_{c}_{t}")
            mm(z_ps[:, 0:CH], w1t[0:64, :], xt_c[0:64, :], start=True, stop=True,
                             tile_position=(0, 0))
            mm(z_ps[:, CH : 2 * CH], w1t[64:128, :], xt_c[64:128, :], start=True, stop=True,
                             tile_position=(64, 0))
            z_sb = apool.tile([128, 2 * CH], BF16, tag="zsb", name=f"zsb_{c}_{t}")
            nc.vector.tensor_scalar_max(z_sb[:], z_ps[:], 0.0)
            return z_sb

        def emit_L4(c, t, g_sb):
            # L4 transposed: lhsT=g (M=128 batch cols), rhs=w4 (N=2) ->
            # out [b, node] with nodes on the PSUM free axis
            if c not in obanks:
                obanks[c] = (
                    ps_o.tile([128, 4 * N], F32, tag="o", name=f"o_{c}"),
                    apool.tile([128, 4 * N], F32, tag="osb", name=f"osb_{c}"))
            o_bank, o_sb = obanks[c]
            w4t = w4_sb[:, t * 2 : (t + 1) * 2]
            for bb in range(4):
                mm(
                    o_bank[:, bb * N + 2 * t : bb * N + 2 * t + 2],
                    g_sb[:, bb * 128 : (bb + 1) * 128],
                    w4t[:],
                    start=True, stop=True)
            if t == npair - 1:
                nc.scalar.activation(o_sb[:], o_bank[:], Copy)
                nc.sync.dma_start(
                    out_d[c * CH : (c + 1) * CH, :].rearrange(
                        "(bb p) n -> p bb n", p=128),
                    o_sb[:].rearrange("p (bb n) -> p bb n", n=N))

        z_cur = {0: emit_L1(0, siters[0][1]), 1: emit_L1(0, siters[0][2])}
        pend_L4 = []  # up to 2 of (c, t, g_sb) awaiting emission
        for s, (c, a, b) in enumerate(siters):
            cs = slice(c * CH, (c + 1) * CH)
            # x+bias window: one K=3 mm per pair (rows 4t..4t+2 of xt4 =
            # [x_2t, x_2t+1, ones]); pairs a,b live in disjoint 32-row
            # strips so both mms stream in one PE window.  Starts both
            # halves of the h accumulation; mains stop them.
            h_ps = ps_h.tile([128, 2 * CH], F32, tag="h", name=f"h_{s}")
            for half, t in enumerate((a, b)):
                base = 32 * (t % 4)
                xoff = (t // 4) * bl + c * CH
                mm(h_ps[:, half * CH : (half + 1) * CH],
                   w3xb_sb[base : base + 3, t * 128 : (t + 1) * 128],
                   xt4_sb[base : base + 3, xoff : xoff + CH],
                   start=True, stop=False, skip_group_check=True,
                   tile_position=(base, 0))

            for half, t in enumerate((a, b)):
                z_sb = z_cur[half]
                w2t = w2_sb[:, t * 128 : (t + 1) * 128]
                w3t = w3_sb[:, t * 128 : (t + 1) * 128]

                # L2: block-diag (K=128, M=64) per node, packed into one PSUM
                r_ps = ps_r.tile([128, CH], F32, tag="r")
                mm(r_ps[0:64, :], w2t[:, 0:64], z_sb[:, 0:CH], start=True, stop=True,
                                 tile_position=(0, 0))
                mm(r_ps[64:128, :], w2t[:, 64:128], z_sb[:, CH : 2 * CH], start=True, stop=True,
                                 tile_position=(0, 64))
                f_sb = apool.tile([128, CH], BF16, tag="f")
                nc.scalar.activation(f_sb[:], r_ps[:], Relu)

                # L1 for the same slot of the next super-iteration (fills
                # the PE while f is on ACT)
                if s + 1 < len(siters):
                    nt = siters[s + 1][1 + half]
                    z_cur[half] = emit_L1(siters[s + 1][0], nt)

                # L4 of the pair two halves ago: reads g two generations
                # back, must be emitted before this half's g-relu reuses
                # that buffer
                if len(pend_L4) >= 2:
                    emit_L4(*pend_L4.pop(0))

                # L3 main: single K=128 M=128 mm on the block-diagonal w3
                # (off-diag blocks zero); stops this half's accumulation
                mm(h_ps[:, half * CH : (half + 1) * CH], w3t[:, :], f_sb[:, :],
                   start=False, stop=True, skip_group_check=True,
                   tile_position=(0, 0))
                g_sb = apool.tile([128, CH], BF16, tag="g")
                nc.scalar.activation(g_sb[:], h_ps[:, half * CH : (half + 1) * CH], Relu)
                pend_L4.append((c, t, g_sb))
        while pend_L4:
            emit_L4(*pend_L4.pop(0))

    nc.compile()
    return nc


def _prep_weights(W1a, W1b, W2a, W2b, W3a, b3a, W3b, npair=NPAIR):
    import ml_dtypes
    n = W1a.shape[0]
    mask = (1.0 - np.eye(n, dtype=np.float32))  # [i, n]
    W1am = W1a * mask[:, None, :]
    W2am = W2a * mask[:, None, :]
    w1 = np.zeros((npair, 128, 128), np.float32)
    w2 = np.zeros((npair, 128, 128), np.float32)
    w3 = np.zeros((npair, 128, 128), np.float32)
    w4 = np.zeros((npair, 128, 2), np.float32)
    w3xb = np.zeros((128, npair * 128), np.float32)
    for t in range(npair):
        i0, i1 = 2 * t, 2 * t + 1
        w1[t, 0:64, 0:64] = W1am[i0].T
        w1[t, 0:64, 64:128] = W2am[i0].T
        w1[t, 64:128, 0:64] = W1am[i1].T
        w1[t, 64:128, 64:128] = W2am[i1].T
        w2[t, 0:64, 0:32] = W1b[i0].T
        w2[t, 64:128, 32:64] = W2b[i0].T
        w2[t, 0:64, 64:96] = W1b[i1].T
        w2[t, 64:128, 96:128] = W2b[i1].T
        w3[t, 0:64, 0:64] = W3a[i0][:, 0:64].T
        w3[t, 64:128, 64:128] = W3a[i1][:, 0:64].T
        # x-passthrough + bias as a K=3 lhsT at 32-aligned partition base
        base = 32 * (t % 4)
        w3xb[base + 0, t * 128 : t * 128 + 64] = W3a[i0][:, 64 + i0]
        w3xb[base + 1, t * 128 + 64 : t * 128 + 128] = W3a[i1][:, 64 + i1]
        w3xb[base + 2, t * 128 : t * 128 + 64] = b3a[i0]
        w3xb[base + 2, t * 128 + 64 : t * 128 + 128] = b3a[i1]
        w4[t, 0:64, 0] = W3b[i0, 0]
        w4[t, 64:128, 1] = W3b[i1, 0]
    # pack pair-major arrays into the SBUF layout [P, npair*F], bf16
    pk = lambda a: np.ascontiguousarray(
        a.transpose(1, 0, 2).reshape(a.shape[1], -1)).astype(ml_dtypes.bfloat16)
    return pk(w1), pk(w2), pk(w3), w3xb.astype(ml_dtypes.bfloat16), pk(w4)


def _make_in_maps(inputs):
    """FULL input dict -> per-core in_maps list (host-side pack/shard)."""
    import ml_dtypes
    x = np.asarray(inputs["x"], np.float32)
    w1, w2, w3, w3xb, w4 = _prep_weights(
        np.asarray(inputs["W1a"], np.float32), np.asarray(inputs["W1b"], np.float32),
        np.asarray(inputs["W2a"], np.float32), np.asarray(inputs["W2b"], np.float32),
        np.asarray(inputs["W3a"], np.float32), np.asarray(inputs["b3a"], np.float32),
        np.asarray(inputs["W3b"], np.float32))
    in_maps = []
    for core in range(NCORES):
        xs = x[core * BL : (core + 1) * BL]            # [BL, 64]
        xt = np.ascontiguousarray(
            np.concatenate([xs.T, xs.T], axis=0)).astype(ml_dtypes.bfloat16)
        xt4 = np.zeros((96, BL), np.float32)           # compact: 3 rows/pair
        xt4[0::3] = xs.T[0::2]                         # x_{2t}
        xt4[1::3] = xs.T[1::2]                         # x_{2t+1}
        xt4[2::3] = 1.0                                # ones (bias row)
        xt4 = xt4.astype(ml_dtypes.bfloat16)
        in_maps.append({"xt": xt, "xt4": xt4, "w1": w1, "w2": w2, "w3": w3,
                        "w3xb": w3xb, "w4": w4})
    return in_maps


def kernel(x, W1a, W1b, W2a, W2b, W3a, b3a, W3b, b3b):
    from concourse.bass_utils import run_bass_kernel_spmd

    inputs = {"x": x, "W1a": W1a, "W1b": W1b, "W2a": W2a, "W2b": W2b,
              "W3a": W3a, "b3a": b3a, "W3b": W3b}
    in_maps = _make_in_maps(inputs)
    b3b = np.asarray(b3b, np.float32)

    if "nc" not in _cache:
        _cache["nc"] = _build_bass(BL, NPAIR)
    nc = _cache["nc"]

    res = run_bass_kernel_spmd(nc, in_maps, core_ids=list(range(NCORES)))
    out = np.empty((B, N), np.float32)
    for core in range(NCORES):
        opre = res.results[core]["opre"]               # [BL, 64]
        out[core * BL : (core + 1) * BL] = np.maximum(opre + b3b[:, 0][None, :], 0.0)
    return out


# revision 14
# speedup vs baseline: 1.0783x; 1.0783x over previous
"""Trainium2 Bass kernel for nn_CausalTrajectoryPrediction.

Per-node stacked MLP over B=16384 rows, N=64 nodes:
  x1[b,i,:] = x[b,:] with entry i zeroed       (mask folded into weights host-side)
  z_i  = relu(W1a'_i @ x) , relu(W2a'_i @ x)   (two branches, packed M=128)
  r_i  = relu(blockdiag(W1b_i, W2b_i) @ z_i)   (K=128, M=64)
  h_i  = relu(W3ab_i @ r_i + w3x_i * x[:,i] + b3a_i)
  out  = relu(w3b_i . h_i + b3b_i)             (final bias+relu on host)

Layout: activations transposed [feature, B]; batch sharded across 8 cores
(BL=2048 each); nodes processed in pairs so every ACT/DVE op uses 128
partitions; matmul groups are subarray-tiled via tile_position for PE
concurrency.  Inputs arrive as prepacked DRAM tensors (host does all
transposes/masking); built on Bacc so multi-semaphore waits are split
into EventSemaphores (walrus allows one wait per Matmult).

Perf notes (measured on 8x trn2; 868us fp32 -> 247us bf16 -> this):
- all matmul operands bf16 (1 PE cycle/row vs 4 for fp32); PSUM stays
  fp32.  fp8 is out: e4m3 on L1 alone already gives 2.7e-2 rel err.
- row-disjoint matmul pairs share one N=512 stream window on the PE.
- x-passthrough AND b3a are folded into ONE K=3 matmul per pair via
  xt4 (host-packed [x_2t, x_2t+1, ones, 0] partition quads) and w3xb
  (lhsT rows = [w3x_i0|0], [0|w3x_i1], [b3a_i0|b3a_i1]).  Pairs t and
  t+16 sit in disjoint 32-row strips of xt4, so their K=3 mms share
  one stream window (x+bias cost halves vs one window per pair, and
  the ACT g-relu loses its bias read).
- super-iteration (t, t+16): h is one [128,1024] PSUM tile; per-pair
  g-relus read halves.  PSUM banks: z 4 + r 1 + h 2 + o 1 = 8 (full).
- L4(prev) is issued before main/g of the current pair: it depends on
  an ACT output (g), so issuing it late avoids head-of-line-blocking
  the in-order PE queue; z relu is one merged [128,1024] DVE op.
- Anti-diagonal tile positions (64,0)/(0,64) crash at runtime (HW
  quadrant bug); GPSIMD cannot read PSUM, so relus stay on ACT/DVE.
"""

import numpy as np
from contextlib import ExitStack

N, H, M, B = 64, 64, 32, 16384
NCORES = 8
BL = B // NCORES            # 2048 batch columns per core
CH = 512                    # chunk width (one PSUM bank of fp32)
NPAIR = N // 2              # 32 node pairs

_cache = {}


def _build_bass(bl, npair):
    import concourse.bass as bass
    import concourse.bacc as bacc
    import concourse.mybir as mybir
    import concourse.tile as tile

    F32 = mybir.dt.float32
    BF16 = mybir.dt.bfloat16
    Relu = mybir.ActivationFunctionType.Relu
    Copy = mybir.ActivationFunctionType.Copy
    nch = bl // CH

    nc = bacc.Bacc()
    xt_d = nc.dram_tensor("xt", [128, bl], BF16, kind="ExternalInput")
    xt4_d = nc.dram_tensor("xt4", [96, bl], BF16, kind="ExternalInput")
    w1_d = nc.dram_tensor("w1", [128, npair * 128], BF16, kind="ExternalInput")
    w2_d = nc.dram_tensor("w2", [128, npair * 128], BF16, kind="ExternalInput")
    w3_d = nc.dram_tensor("w3", [128, npair * 128], BF16, kind="ExternalInput")
    w3xb_d = nc.dram_tensor("w3xb", [128, npair * 128], BF16, kind="ExternalInput")
    w4_d = nc.dram_tensor("w4", [128, npair * 2], BF16, kind="ExternalInput")
    out_d = nc.dram_tensor("opre", [bl, N], F32, kind="ExternalOutput")

    mm = nc.tensor.matmul  # bf16 operands: 1 PE cycle/row (fp32 was 4)

    with tile.TileContext(nc) as tc, ExitStack() as ctx:
        wpool = ctx.enter_context(tc.tile_pool(name="weights", bufs=1))
        apool = ctx.enter_context(tc.tile_pool(name="acts", bufs=2))
        ps_z = ctx.enter_context(tc.tile_pool(name="ps_z", bufs=2, space="PSUM"))
        ps_r = ctx.enter_context(tc.tile_pool(name="ps_r", bufs=1, space="PSUM"))
        ps_h = ctx.enter_context(tc.tile_pool(name="ps_h", bufs=1, space="PSUM"))
        ps_o = ctx.enter_context(tc.tile_pool(name="ps_o", bufs=1, space="PSUM"))

        # Parallel HWDGE loads; xt/w1 first so L1 compute starts ASAP.
        xt_sb = wpool.tile([128, bl], BF16, tag="xt")
        nc.sync.dma_start(xt_sb[:, 0:CH], xt_d[:, 0:CH])
        w1_sb = wpool.tile([128, npair * 128], BF16, tag="w1")
        nc.sync.dma_start(w1_sb[:, 0:512], w1_d[:, 0:512])
        nc.sync.dma_start(xt_sb[:, CH:bl], xt_d[:, CH:bl])
        nc.sync.dma_start(w1_sb[:, 512 : npair * 128], w1_d[:, 512 : npair * 128])
        w2_sb = wpool.tile([128, npair * 128], BF16, tag="w2")
        nc.sync.dma_start(w2_sb[:, 0:512], w2_d[:, 0:512])
        nc.sync.dma_start(w2_sb[:, 512 : npair * 128], w2_d[:, 512 : npair * 128])
        # xt4: matmul APs must start at 32-aligned partitions, so pair t's
        # [x_2t, x_2t+1, ones] rows land at partition 32*(t%4) in batch-slot
        # t//4 (8 slots along the free axis), scattered from a compact
        # [96, bl] DRAM tensor
        xt4_sb = wpool.tile([128, 8 * bl], BF16, tag="xt4")
        for t in range(npair):
            nc.sync.dma_start(
                xt4_sb[32 * (t % 4) : 32 * (t % 4) + 3,
                       (t // 4) * bl : (t // 4) * bl + bl],
                xt4_d[3 * t : 3 * t + 3, :])
        w3_sb = wpool.tile([128, npair * 128], BF16, tag="w3")
        nc.sync.dma_start(w3_sb[:, 0:512], w3_d[:, 0:512])
        nc.sync.dma_start(w3_sb[:, 512 : npair * 128], w3_d[:, 512 : npair * 128])
        w3xb_sb = wpool.tile([128, npair * 128], BF16, tag="w3xb")
        nc.sync.dma_start(w3xb_sb[:], w3xb_d[:])
        w4_sb = wpool.tile([128, npair * 2], BF16, tag="w4")
        nc.sync.dma_start(w4_sb[:], w4_d[:])

        iters = [(c, t) for c in range(nch) for t in range(npair)]
        obanks = {}

        # PE warm-up: ~3.5us of junk matmuls on the first xt chunk while the
        # weight DMAs stream in, so HAM un-throttles (1.2 -> 2.4 GHz) before
        # the real loop starts instead of ~30us into it.  Output is discarded.
        warm_ps = ps_h.tile([128, CH], F32, tag="h", name="warm")
        for _ in range(10):
            # sequential full-array overwrites of one scratch bank (a
            # concurrent row-tiled pair here would WW-collide on the bank)
            mm(warm_ps[:], xt_sb[0:64, 0:128], xt_sb[0:64, 0:CH],
               start=True, stop=True, tile_position=(0, 0))

        def emit_L1(c, t):
            # L1: both branches for each node of the pair (K=64, M=128);
            # the two nodes run row-concurrent on the PE (xt duplicated at
            # partitions 64-127).  z relu split in two DVE ops: L2-mm1's
            # half is ready at ~half the latency and the z-bank WAR frees
            # earlier (a single merged relu staggered the next L1 pair).
            xt_c = xt_sb[:, c * CH : (c + 1) * CH]
            w1t = w1_sb[:, t * 128 : (t + 1) * 128]
            z_ps = ps_z.tile([128, 2 * CH], F32, tag="z", name=f"z_{c}_{t}")
            mm(z_ps[:, 0:CH], w1t[0:64, :], xt_c[0:64, :], start=True, stop=True,
                             tile_position=(0, 0))
            mm(z_ps[:, CH : 2 * CH], w1t[64:128, :], xt_c[64:128, :], start=True, stop=True,
                             tile_position=(64, 0))
            z_sb = apool.tile([128, 2 * CH], BF16, tag="zsb", name=f"zsb_{c}_{t}")
            nc.vector.tensor_scalar_max(z_sb[:, 0:CH], z_ps[:, 0:CH], 0.0)
            nc.vector.tensor_scalar_max(z_sb[:, CH : 2 * CH], z_ps[:, CH : 2 * CH], 0.0)
            return z_sb

        def emit_L4(c, t, g_sb):
            # L4 transposed: lhsT=g (M=128 batch cols), rhs=w4 (N=2) ->
            # out [b, node] with nodes on the PSUM free axis
            if c not in obanks:
                obanks[c] = (
                    ps_o.tile([128, 4 * N], F32, tag="o", name=f"o_{c}"),
                    apool.tile([128, 4 * N], F32, tag="osb", name=f"osb_{c}"))
            o_bank, o_sb = obanks[c]
            w4t = w4_sb[:, t * 2 : (t + 1) * 2]
            for bb in range(4):
                mm(
                    o_bank[:, bb * N + 2 * t : bb * N + 2 * t + 2],
                    g_sb[:, bb * 128 : (bb + 1) * 128],
                    w4t[:],
                    start=True, stop=True)
            if t == npair - 1:
                nc.scalar.activation(o_sb[:], o_bank[:], Copy)
                nc.sync.dma_start(
                    out_d[c * CH : (c + 1) * CH, :].rearrange(
                        "(bb p) n -> p bb n", p=128),
                    o_sb[:].rearrange("p (bb n) -> p bb n", n=N))

        z_cur = emit_L1(*iters[0])
        pend_L4 = None  # (c, t, g_sb) awaiting emission
        for k, (c, t) in enumerate(iters):
            z_sb = z_cur
            w2t = w2_sb[:, t * 128 : (t + 1) * 128]
            w3t = w3_sb[:, t * 128 : (t + 1) * 128]

            # L2: block-diag (K=128, M=64) per node, packed into one PSUM
            r_ps = ps_r.tile([128, CH], F32, tag="r")
            mm(r_ps[0:64, :], w2t[:, 0:64], z_sb[:, 0:CH], start=True, stop=True,
                             tile_position=(0, 0))
            mm(r_ps[64:128, :], w2t[:, 64:128], z_sb[:, CH : 2 * CH], start=True, stop=True,
                             tile_position=(0, 64))
            f_sb = apool.tile([128, CH], BF16, tag="f")
            nc.scalar.activation(f_sb[:], r_ps[:], Relu)

            # L3: the x-passthrough AND b3a arrive via ONE K=3 mm (rows
            # [x_2t, x_2t+1, ones] of xt4) that starts the accumulation;
            # the main is a single K=128 M=128 mm on the block-diagonal w3
            # (off-diag blocks zero) that stops it.
            h_ps = ps_h.tile([128, CH], F32, tag="h")
            base = 32 * (t % 4)
            xoff = (t // 4) * bl + c * CH
            mm(h_ps[:, :],
               w3xb_sb[base : base + 3, t * 128 : (t + 1) * 128],
               xt4_sb[base : base + 3, xoff : xoff + CH],
               start=True, stop=False, skip_group_check=True,
               tile_position=(base, 0))
            mm(h_ps[:, :], w3t[:, :], f_sb[:, :],
               start=False, stop=True, skip_group_check=True,
               tile_position=(0, 0))
            g_sb = apool.tile([128, CH], BF16, tag="g")
            nc.scalar.activation(g_sb[:], h_ps[:], Relu)

            if k + 1 < len(iters):
                z_cur = emit_L1(*iters[k + 1])
            # L4(k-1) is emitted AFTER L1(k+1): it depends on g(k-1) (an ACT
            # output), so issuing it earlier head-of-line-blocks the in-order
            # PE queue (~200ns/iter measured)
            if pend_L4 is not None:
                emit_L4(*pend_L4)
            pend_L4 = (c, t, g_sb)
        emit_L4(*pend_L4)

    nc.compile()
    return nc


def _prep_weights(W1a, W1b, W2a, W2b, W3a, b3a, W3b, npair=NPAIR):
    import ml_dtypes
    n = W1a.shape[0]
    mask = (1.0 - np.eye(n, dtype=np.float32))  # [i, n]
    W1am = W1a * mask[:, None, :]
    W2am = W2a * mask[:, None, :]
    w1 = np.zeros((npair, 128, 128), np.float32)
    w2 = np.zeros((npair, 128, 128), np.float32)
    w3 = np.zeros((npair, 128, 128), np.float32)
    w4 = np.zeros((npair, 128, 2), np.float32)
    w3xb = np.zeros((128, npair * 128), np.float32)
    for t in range(npair):
        i0, i1 = 2 * t, 2 * t + 1
        w1[t, 0:64, 0:64] = W1am[i0].T
        w1[t, 0:64, 64:128] = W2am[i0].T
        w1[t, 64:128, 0:64] = W1am[i1].T
        w1[t, 64:128, 64:128] = W2am[i1].T
        w2[t, 0:64, 0:32] = W1b[i0].T
        w2[t, 64:128, 32:64] = W2b[i0].T
        w2[t, 0:64, 64:96] = W1b[i1].T
        w2[t, 64:128, 96:128] = W2b[i1].T
        w3[t, 0:64, 0:64] = W3a[i0][:, 0:64].T
        w3[t, 64:128, 64:128] = W3a[i1][:, 0:64].T
        # x-passthrough + bias as a K=3 lhsT at 32-aligned partition base
        base = 32 * (t % 4)
        w3xb[base + 0, t * 128 : t * 128 + 64] = W3a[i0][:, 64 + i0]
        w3xb[base + 1, t * 128 + 64 : t * 128 + 128] = W3a[i1][:, 64 + i1]
        w3xb[base + 2, t * 128 : t * 128 + 64] = b3a[i0]
        w3xb[base + 2, t * 128 + 64 : t * 128 + 128] = b3a[i1]
        w4[t, 0:64, 0] = W3b[i0, 0]
        w4[t, 64:128, 1] = W3b[i1, 0]
    # pack pair-major arrays into the SBUF layout [P, npair*F], bf16
    pk = lambda a: np.ascontiguousarray(
        a.transpose(1, 0, 2).reshape(a.shape[1], -1)).astype(ml_dtypes.bfloat16)
    return pk(w1), pk(w2), pk(w3), w3xb.astype(ml_dtypes.bfloat16), pk(w4)


def _make_in_maps(inputs):
    """FULL input dict -> per-core in_maps list (host-side pack/shard)."""
    import ml_dtypes
    x = np.asarray(inputs["x"], np.float32)
    w1, w2, w3, w3xb, w4 = _prep_weights(
        np.asarray(inputs["W1a"], np.float32), np.asarray(inputs["W1b"], np.float32),
        np.asarray(inputs["W2a"], np.float32), np.asarray(inputs["W2b"], np.float32),
        np.asarray(inputs["W3a"], np.float32), np.asarray(inputs["b3a"], np.float32),
        np.asarray(inputs["W3b"], np.float32))
    in_maps = []
    for core in range(NCORES):
        xs = x[core * BL : (core + 1) * BL]            # [BL, 64]
        xt = np.ascontiguousarray(
            np.concatenate([xs.T, xs.T], axis=0)).astype(ml_dtypes.bfloat16)
        xt4 = np.zeros((96, BL), np.float32)           # compact: 3 rows/pair
        xt4[0::3] = xs.T[0::2]                         # x_{2t}
        xt4[1::3] = xs.T[1::2]                         # x_{2t+1}
        xt4[2::3] = 1.0                                # ones (bias row)
        xt4 = xt4.astype(ml_dtypes.bfloat16)
        in_maps.append({"xt": xt, "xt4": xt4, "w1": w1, "w2": w2, "w3": w3,
                        "w3xb": w3xb, "w4": w4})
    return in_maps


def kernel(x, W1a, W1b, W2a, W2b, W3a, b3a, W3b, b3b):
    from concourse.bass_utils import run_bass_kernel_spmd

    inputs = {"x": x, "W1a": W1a, "W1b": W1b, "W2a": W2a, "W2b": W2b,
              "W3a": W3a, "b3a": b3a, "W3b": W3b}
    in_maps = _make_in_maps(inputs)
    b3b = np.asarray(b3b, np.float32)

    if "nc" not in _cache:
        _cache["nc"] = _build_bass(BL, NPAIR)
    nc = _cache["nc"]

    res = run_bass_kernel_spmd(nc, in_maps, core_ids=list(range(NCORES)))
    out = np.empty((B, N), np.float32)
    for core in range(NCORES):
        opre = res.results[core]["opre"]               # [BL, 64]
        out[core * BL : (core + 1) * BL] = np.maximum(opre + b3b[:, 0][None, :], 0.0)
    return out


# revision 17
# speedup vs baseline: 1.4363x; 1.3321x over previous
"""Trainium2 Bass kernel for nn_CausalTrajectoryPrediction.

Per-node stacked MLP over B=16384 rows, N=64 nodes:
  x1[b,i,:] = x[b,:] with entry i zeroed       (mask folded into weights host-side)
  z_i  = relu(W1a'_i @ x) , relu(W2a'_i @ x)   (two branches, packed M=128)
  r_i  = relu(blockdiag(W1b_i, W2b_i) @ z_i)   (K=128, M=64)
  h_i  = relu(W3ab_i @ r_i + w3x_i * x[:,i] + b3a_i)
  out  = relu(w3b_i . h_i + b3b_i)             (final bias+relu on host)

Layout: activations transposed [feature, B]; batch sharded across 8 cores
(BL=2048 each); nodes processed in pairs so every ACT/DVE op uses 128
partitions; matmul groups are subarray-tiled via tile_position for PE
concurrency.  Inputs arrive as 7 prepacked DRAM tensors (host does all
transposes/masking); built on Bacc so multi-semaphore waits are split
into EventSemaphores (walrus allows one wait per Matmult).

Perf notes (measured, 8x trn2, ~247us vs 868us fp32 baseline):
- all matmul operands bf16 (1 PE cycle/row vs 4 for fp32; fp32r is also
  1 cyc/row but requires dst base_partition 0, which kills the offset-64
  packing used here).  PSUM accumulation stays fp32; rel err 3.4e-3.
- row-disjoint matmul pairs share one N=512 stream window on the PE
  (L1: rows 0-63 / 64-127; L2: col halves; measured pair ~= single mm).
- L3 runs as 4 mms in 2 windows: the x-passthrough terms start the
  PSUM accumulation from DIAGONAL 64x64 quadrants (0,0)/(64,64) —
  depending only on xt, they issue early — and the f-mains stop it.
  Anti-diagonal positions (64,0)/(0,64) crash at runtime (HW quadrant
  bug); GPSIMD cannot read PSUM, so relus stay on ACT/DVE.
"""

import numpy as np
from contextlib import ExitStack

N, H, M, B = 64, 64, 32, 16384
NCORES = 8
BL = B // NCORES            # 2048 batch columns per core
CH = 512                    # chunk width (one PSUM bank of fp32)
NPAIR = N // 2              # 32 node pairs

_cache = {}


def _build_bass(bl, npair):
    import concourse.bass as bass
    import concourse.bacc as bacc
    import concourse.mybir as mybir
    import concourse.tile as tile

    F32 = mybir.dt.float32
    BF16 = mybir.dt.bfloat16
    Relu = mybir.ActivationFunctionType.Relu
    Copy = mybir.ActivationFunctionType.Copy
    nch = bl // CH

    nc = bacc.Bacc()
    xt_d = nc.dram_tensor("xt", [128, bl], BF16, kind="ExternalInput")
    w1_d = nc.dram_tensor("w1", [128, npair * 128], BF16, kind="ExternalInput")
    w2_d = nc.dram_tensor("w2", [128, npair * 128], BF16, kind="ExternalInput")
    w3_d = nc.dram_tensor("w3", [128, npair * 128], BF16, kind="ExternalInput")
    w3x_d = nc.dram_tensor("w3x", [128, npair * 128], BF16, kind="ExternalInput")
    w4_d = nc.dram_tensor("w4", [128, npair * 2], BF16, kind="ExternalInput")
    b3a_d = nc.dram_tensor("b3a", [128, npair], F32, kind="ExternalInput")
    out_d = nc.dram_tensor("opre", [bl, N], F32, kind="ExternalOutput")

    mm = nc.tensor.matmul  # bf16 operands: 1 PE cycle/row (fp32 was 4)

    with tile.TileContext(nc) as tc, ExitStack() as ctx:
        wpool = ctx.enter_context(tc.tile_pool(name="weights", bufs=1))
        apool = ctx.enter_context(tc.tile_pool(name="acts", bufs=2))
        ps_z = ctx.enter_context(tc.tile_pool(name="ps_z", bufs=2, space="PSUM"))
        ps_r = ctx.enter_context(tc.tile_pool(name="ps_r", bufs=2, space="PSUM"))
        ps_h = ctx.enter_context(tc.tile_pool(name="ps_h", bufs=1, space="PSUM"))
        ps_o = ctx.enter_context(tc.tile_pool(name="ps_o", bufs=1, space="PSUM"))

        # Parallel HWDGE loads; xt/w1 first so L1 compute starts ASAP.
        # (Bacc's generate_event_semaphores splits multi-waits, so matmuls
        # may depend on several DMA queues safely.)
        xt_sb = wpool.tile([128, bl], BF16, tag="xt")
        nc.sync.dma_start(xt_sb[:, 0:CH], xt_d[:, 0:CH])
        w1_sb = wpool.tile([128, npair * 128], BF16, tag="w1")
        nc.sync.dma_start(w1_sb[:, 0:512], w1_d[:, 0:512])
        nc.sync.dma_start(xt_sb[:, CH:bl], xt_d[:, CH:bl])
        nc.sync.dma_start(w1_sb[:, 512 : npair * 128], w1_d[:, 512 : npair * 128])
        w2_sb = wpool.tile([128, npair * 128], BF16, tag="w2")
        nc.sync.dma_start(w2_sb[:, 0:512], w2_d[:, 0:512])
        nc.sync.dma_start(w2_sb[:, 512 : npair * 128], w2_d[:, 512 : npair * 128])
        w3_sb = wpool.tile([128, npair * 128], BF16, tag="w3")
        nc.sync.dma_start(w3_sb[:, 0:512], w3_d[:, 0:512])
        nc.sync.dma_start(w3_sb[:, 512 : npair * 128], w3_d[:, 512 : npair * 128])
        w3x_sb = wpool.tile([128, npair * 128], BF16, tag="w3x")
        nc.sync.dma_start(w3x_sb[:], w3x_d[:])
        w4_sb = wpool.tile([128, npair * 2], BF16, tag="w4")
        nc.sync.dma_start(w4_sb[:], w4_d[:])
        b3a_sb = wpool.tile([128, npair], F32, tag="b3a")
        nc.sync.dma_start(b3a_sb[:], b3a_d[:])

        iters = [(c, t) for c in range(nch) for t in range(npair)]
        obanks = {}

        # PE warm-up: ~6us of sequential junk matmuls on the first xt chunk
        # while the weight DMAs stream in, so HAM un-throttles (1.2 ->
        # 2.4 GHz) before the loop instead of ~30us into it.  Sequential
        # full-array overwrites of one scratch bank (a concurrent row-tiled
        # pair here would WW-collide on the bank).  Output is never read.
        warm_ps = ps_h.tile([128, CH], F32, tag="h", name="warm")
        for _ in range(16):
            mm(warm_ps[:], xt_sb[0:64, 0:128], xt_sb[0:64, 0:CH],
               start=True, stop=True, tile_position=(0, 0))

        def emit_L1(c, t):
            # L1: both branches for each node of the pair (K=64, M=128);
            # the two nodes run row-concurrent on the PE (xt duplicated at
            # partitions 64-127)
            xt_c = xt_sb[:, c * CH : (c + 1) * CH]
            w1t = w1_sb[:, t * 128 : (t + 1) * 128]
            z_ps = ps_z.tile([128, 2 * CH], F32, tag="z", name=f"z_{c}_{t}")
            mm(z_ps[:, 0:CH], w1t[0:64, :], xt_c[0:64, :], start=True, stop=True,
                             tile_position=(0, 0))
            mm(z_ps[:, CH : 2 * CH], w1t[64:128, :], xt_c[64:128, :], start=True, stop=True,
                             tile_position=(64, 0))
            z_sb = apool.tile([128, 2 * CH], BF16, tag="zsb", name=f"zsb_{c}_{t}")
            # one merged relu [128, 2CH]: fewer DVE instructions (throughput
            # beats the split's latency win now that DVE is near-saturated)
            nc.vector.tensor_scalar_max(z_sb[:], z_ps[:], 0.0)
            return z_sb

        def emit_L4(c, t, g_sb):
            # L4 transposed: lhsT=g (M=128 batch cols), rhs=w4 (N=2) ->
            # out [b, node] with nodes on the PSUM free axis
            if c not in obanks:
                obanks[c] = (
                    ps_o.tile([128, 4 * N], F32, tag="o", name=f"o_{c}"),
                    apool.tile([128, 4 * N], F32, tag="osb", name=f"osb_{c}"))
            o_bank, o_sb = obanks[c]
            w4t = w4_sb[:, t * 2 : (t + 1) * 2]
            for bb in range(4):
                mm(
                    o_bank[:, bb * N + 2 * t : bb * N + 2 * t + 2],
                    g_sb[:, bb * 128 : (bb + 1) * 128],
                    w4t[:],
                    start=True, stop=True)
            if t == npair - 1:
                nc.scalar.activation(o_sb[:], o_bank[:], Copy)
                nc.sync.dma_start(
                    out_d[c * CH : (c + 1) * CH, :].rearrange(
                        "(bb p) n -> p bb n", p=128),
                    o_sb[:].rearrange("p (bb n) -> p bb n", n=N))

        z_cur = emit_L1(*iters[0])
        pend_L4 = None  # (c, t, g_sb) awaiting emission
        for k, (c, t) in enumerate(iters):
            cs = slice(c * CH, (c + 1) * CH)
            xt_c = xt_sb[:, cs]
            w2t = w2_sb[:, t * 128 : (t + 1) * 128]
            w3t = w3_sb[:, t * 128 : (t + 1) * 128]
            w3xt2 = w3x_sb[:, t * 128 : (t + 1) * 128]
            z_sb = z_cur

            # L2: block-diag (K=128, M=64) per node, packed into one PSUM
            r_ps = ps_r.tile([128, CH], F32, tag="r")
            mm(r_ps[0:64, :], w2t[:, 0:64], z_sb[:, 0:CH], start=True, stop=True,
                             tile_position=(0, 0))
            mm(r_ps[64:128, :], w2t[:, 64:128], z_sb[:, CH : 2 * CH], start=True, stop=True,
                             tile_position=(0, 64))
            f_sb = apool.tile([128, CH], BF16, tag="f")
            nc.scalar.activation(f_sb[:], r_ps[:], Relu)

            # L3: x2-terms start the accumulation (diagonal 64x64 quadrants,
            # one concurrent window); the main is a single K=128 M=128 mm on
            # the block-diagonal w3 (off-diag blocks are zero) that stops it.
            h_ps = ps_h.tile([128, CH], F32, tag="h")
            mm(h_ps[0:64, :], w3xt2[0:64, 0:64], xt_c[0:64, :], start=True, stop=False, skip_group_check=True,
               tile_position=(0, 0))
            mm(h_ps[64:128, :], w3xt2[64:128, 64:128], xt_c[64:128, :], start=True, stop=False, skip_group_check=True,
               tile_position=(64, 64))
            mm(h_ps[:, :], w3t[:, :], f_sb[:, :], start=False, stop=True, skip_group_check=True,
               tile_position=(0, 0))
            g_sb = apool.tile([128, CH], BF16, tag="g")
            nc.scalar.activation(g_sb[:], h_ps[:], Relu, bias=b3a_sb[:, t : t + 1])

            if k + 1 < len(iters):
                z_cur = emit_L1(*iters[k + 1])
            # L4(k-1) is emitted AFTER L1(k+1): it depends on g(k-1) (an ACT
            # output), so issuing it earlier head-of-line-blocks the in-order
            # PE queue (~200ns/iter measured)
            if pend_L4 is not None:
                emit_L4(*pend_L4)
            pend_L4 = (c, t, g_sb)
        emit_L4(*pend_L4)

    nc.compile()
    return nc


def _prep_weights(W1a, W1b, W2a, W2b, W3a, b3a, W3b, npair=NPAIR):
    import ml_dtypes
    n = W1a.shape[0]
    mask = (1.0 - np.eye(n, dtype=np.float32))  # [i, n]
    W1am = W1a * mask[:, None, :]
    W2am = W2a * mask[:, None, :]
    w1 = np.zeros((npair, 128, 128), np.float32)
    w2 = np.zeros((npair, 128, 128), np.float32)
    w3 = np.zeros((npair, 128, 128), np.float32)
    w3x = np.zeros((npair, 128, 128), np.float32)
    w4 = np.zeros((npair, 128, 2), np.float32)
    b3ap = np.zeros((128, npair), np.float32)
    for t in range(npair):
        i0, i1 = 2 * t, 2 * t + 1
        w1[t, 0:64, 0:64] = W1am[i0].T
        w1[t, 0:64, 64:128] = W2am[i0].T
        w1[t, 64:128, 0:64] = W1am[i1].T
        w1[t, 64:128, 64:128] = W2am[i1].T
        w2[t, 0:64, 0:32] = W1b[i0].T
        w2[t, 64:128, 32:64] = W2b[i0].T
        w2[t, 0:64, 64:96] = W1b[i1].T
        w2[t, 64:128, 96:128] = W2b[i1].T
        w3[t, 0:64, 0:64] = W3a[i0][:, 0:64].T
        w3[t, 64:128, 64:128] = W3a[i1][:, 0:64].T
        w3x[t, i0, 0:64] = W3a[i0][:, 64 + i0]
        w3x[t, 64 + i0, 0:64] = W3a[i0][:, 64 + i0]
        w3x[t, i1, 64:128] = W3a[i1][:, 64 + i1]
        w3x[t, 64 + i1, 64:128] = W3a[i1][:, 64 + i1]
        w4[t, 0:64, 0] = W3b[i0, 0]
        w4[t, 64:128, 1] = W3b[i1, 0]
        b3ap[0:64, t] = b3a[i0]
        b3ap[64:128, t] = b3a[i1]
    # pack pair-major arrays into the SBUF layout [P, npair*F], bf16
    pk = lambda a: np.ascontiguousarray(
        a.transpose(1, 0, 2).reshape(a.shape[1], -1)).astype(ml_dtypes.bfloat16)
    return pk(w1), pk(w2), pk(w3), pk(w3x), pk(w4), b3ap


def _make_in_maps(inputs):
    """FULL input dict -> per-core in_maps list (host-side pack/shard)."""
    import ml_dtypes
    x = np.asarray(inputs["x"], np.float32)
    w1, w2, w3, w3x, w4, b3ap = _prep_weights(
        np.asarray(inputs["W1a"], np.float32), np.asarray(inputs["W1b"], np.float32),
        np.asarray(inputs["W2a"], np.float32), np.asarray(inputs["W2b"], np.float32),
        np.asarray(inputs["W3a"], np.float32), np.asarray(inputs["b3a"], np.float32),
        np.asarray(inputs["W3b"], np.float32))
    in_maps = []
    for core in range(NCORES):
        xs = x[core * BL : (core + 1) * BL]            # [BL, 64]
        xt = np.ascontiguousarray(
            np.concatenate([xs.T, xs.T], axis=0)).astype(ml_dtypes.bfloat16)
        in_maps.append({"xt": xt, "w1": w1, "w2": w2, "w3": w3,
                        "w3x": w3x, "w4": w4, "b3a": b3ap})
    return in_maps


def kernel(x, W1a, W1b, W2a, W2b, W3a, b3a, W3b, b3b):
    from concourse.bass_utils import run_bass_kernel_spmd

    in_maps = _make_in_maps({"x": x, "W1a": W1a, "W1b": W1b, "W2a": W2a,
                             "W2b": W2b, "W3a": W3a, "b3a": b3a, "W3b": W3b})
    b3b = np.asarray(b3b, np.float32)

    if "nc" not in _cache:
        _cache["nc"] = _build_bass(BL, NPAIR)
    nc = _cache["nc"]

    res = run_bass_kernel_spmd(nc, in_maps, core_ids=list(range(NCORES)))
    out = np.empty((B, N), np.float32)
    for core in range(NCORES):
        opre = res.results[core]["opre"]               # [BL, 64]
        out[core * BL : (core + 1) * BL] = np.maximum(opre + b3b[:, 0][None, :], 0.0)
    return out



# revision 19
# speedup vs baseline: 1.7182x; 1.1963x over previous
"""Trainium2 Bass kernel for nn_CausalTrajectoryPrediction.

Per-node stacked MLP over B=16384 rows, N=64 nodes:
  x1[b,i,:] = x[b,:] with entry i zeroed       (mask folded into weights host-side)
  z_i  = relu(W1a'_i @ x) , relu(W2a'_i @ x)   (two branches, packed M=128)
  r_i  = relu(blockdiag(W1b_i, W2b_i) @ z_i)   (K=128, M=64)
  h_i  = relu(W3ab_i @ r_i + w3x_i * x[:,i] + b3a_i)
  out  = relu(w3b_i . h_i + b3b_i)             (final bias+relu on host)

Layout: activations transposed [feature, B]; batch sharded across 8 cores
(BL=2048 each); nodes processed in pairs so every ACT/DVE op uses 128
partitions; matmul groups are subarray-tiled via tile_position for PE
concurrency.  Inputs arrive as 7 prepacked DRAM tensors (host does all
transposes/masking); built on Bacc so multi-semaphore waits are split
into EventSemaphores (walrus allows one wait per Matmult).

Perf notes (measured, 8x trn2, ~233us; 868us fp32 / 247us first bf16):
- all matmul operands bf16 (1 PE cycle/row vs 4 for fp32; fp32r is also
  1 cyc/row but requires dst base_partition 0, which kills the offset-64
  packing used here).  PSUM accumulation stays fp32; rel err 3.4e-3.
  fp8 is out: e4m3 on L1 alone already gives 2.7e-2 > the 2e-2 gate.
- row-disjoint matmul pairs share one N=512 stream window on the PE
  (L1: rows 0-63 / 64-127; L2: col halves; measured pair ~= single mm).
- L3: the x-passthrough terms start the PSUM accumulation from DIAGONAL
  64x64 quadrants (0,0)/(64,64) — depending only on xt, they issue
  early — and ONE K=128 M=128 mm on the block-diagonal w3 stops it.
  Anti-diagonal positions (64,0)/(0,64) crash at runtime (HW quadrant
  bug); GPSIMD cannot read PSUM, so relus stay on ACT/DVE.
- z relu is one merged [128,1024] DVE op; L4(k-1) is emitted AFTER
  L1(k+1) (it depends on ACT's g(k-1), so issuing it earlier
  head-of-line-blocks the in-order PE queue; ~10us total).
- steady state: period ~1553ns/iter = the ACT chain main->g, f->main
  (2x ~690ns ACTIVATE FIFO) with PE ~94% busy.  PE MUST stay >~90%
  busy: HAM only holds 2.4GHz under sustained activity, and variants
  that idled PE ~15% (x+bias folded into K=3 mms with shared windows,
  h/r rebanking) never un-throttled from 1.2GHz and ran 1.6x SLOWER
  (233 -> 371-400us).  An explicit warm-up mm burst before the
  DMA-gated prologue also hurt (278us): burst-then-idle re-throttles.
"""

import numpy as np
from contextlib import ExitStack

N, H, M, B = 64, 64, 32, 16384
NCORES = 8
BL = B // NCORES            # 2048 batch columns per core
CH = 512                    # chunk width (one PSUM bank of fp32)
NPAIR = N // 2              # 32 node pairs

_cache = {}


def _build_bass(bl, npair):
    import concourse.bass as bass
    import concourse.bacc as bacc
    import concourse.mybir as mybir
    import concourse.tile as tile

    F32 = mybir.dt.float32
    BF16 = mybir.dt.bfloat16
    Relu = mybir.ActivationFunctionType.Relu
    Copy = mybir.ActivationFunctionType.Copy
    nch = bl // CH

    nc = bacc.Bacc()
    xt_d = nc.dram_tensor("xt", [128, bl], BF16, kind="ExternalInput")
    w1_d = nc.dram_tensor("w1", [128, npair * 128], BF16, kind="ExternalInput")
    w2_d = nc.dram_tensor("w2", [128, npair * 128], BF16, kind="ExternalInput")
    w3_d = nc.dram_tensor("w3", [128, npair * 128], BF16, kind="ExternalInput")
    w3x_d = nc.dram_tensor("w3x", [128, npair * 128], BF16, kind="ExternalInput")
    w4_d = nc.dram_tensor("w4", [128, npair * 2], BF16, kind="ExternalInput")
    b3a_d = nc.dram_tensor("b3a", [128, npair], F32, kind="ExternalInput")
    out_d = nc.dram_tensor("opre", [bl, N], F32, kind="ExternalOutput")

    mm = nc.tensor.matmul  # bf16 operands: 1 PE cycle/row (fp32 was 4)

    with tile.TileContext(nc) as tc, ExitStack() as ctx:
        wpool = ctx.enter_context(tc.tile_pool(name="weights", bufs=1))
        apool = ctx.enter_context(tc.tile_pool(name="acts", bufs=2))
        ps_z = ctx.enter_context(tc.tile_pool(name="ps_z", bufs=2, space="PSUM"))
        ps_r = ctx.enter_context(tc.tile_pool(name="ps_r", bufs=2, space="PSUM"))
        ps_h = ctx.enter_context(tc.tile_pool(name="ps_h", bufs=1, space="PSUM"))
        ps_o = ctx.enter_context(tc.tile_pool(name="ps_o", bufs=1, space="PSUM"))

        # Parallel HWDGE loads; xt/w1 first so L1 compute starts ASAP.
        # (Bacc's generate_event_semaphores splits multi-waits, so matmuls
        # may depend on several DMA queues safely.)
        xt_sb = wpool.tile([128, bl], BF16, tag="xt")
        nc.sync.dma_start(xt_sb[:, 0:CH], xt_d[:, 0:CH])
        w1_sb = wpool.tile([128, npair * 128], BF16, tag="w1")
        nc.sync.dma_start(w1_sb[:, 0:512], w1_d[:, 0:512])
        nc.sync.dma_start(xt_sb[:, CH:bl], xt_d[:, CH:bl])
        nc.sync.dma_start(w1_sb[:, 512 : npair * 128], w1_d[:, 512 : npair * 128])
        w2_sb = wpool.tile([128, npair * 128], BF16, tag="w2")
        nc.sync.dma_start(w2_sb[:, 0:512], w2_d[:, 0:512])
        nc.sync.dma_start(w2_sb[:, 512 : npair * 128], w2_d[:, 512 : npair * 128])
        w3_sb = wpool.tile([128, npair * 128], BF16, tag="w3")
        nc.sync.dma_start(w3_sb[:, 0:512], w3_d[:, 0:512])
        nc.sync.dma_start(w3_sb[:, 512 : npair * 128], w3_d[:, 512 : npair * 128])
        w3x_sb = wpool.tile([128, npair * 128], BF16, tag="w3x")
        nc.sync.dma_start(w3x_sb[:], w3x_d[:])
        w4_sb = wpool.tile([128, npair * 2], BF16, tag="w4")
        nc.sync.dma_start(w4_sb[:], w4_d[:])
        b3a_sb = wpool.tile([128, npair], F32, tag="b3a")
        nc.sync.dma_start(b3a_sb[:], b3a_d[:])

        iters = [(c, t) for c in range(nch) for t in range(npair)]
        obanks = {}

        def emit_L1(c, t):
            # L1: both branches for each node of the pair (K=64, M=128);
            # the two nodes run row-concurrent on the PE (xt duplicated at
            # partitions 64-127)
            xt_c = xt_sb[:, c * CH : (c + 1) * CH]
            w1t = w1_sb[:, t * 128 : (t + 1) * 128]
            z_ps = ps_z.tile([128, 2 * CH], F32, tag="z", name=f"z_{c}_{t}")
            mm(z_ps[:, 0:CH], w1t[0:64, :], xt_c[0:64, :], start=True, stop=True,
                             tile_position=(0, 0))
            mm(z_ps[:, CH : 2 * CH], w1t[64:128, :], xt_c[64:128, :], start=True, stop=True,
                             tile_position=(64, 0))
            z_sb = apool.tile([128, 2 * CH], BF16, tag="zsb", name=f"zsb_{c}_{t}")
            # one merged relu [128, 2CH]: fewer DVE instructions (throughput
            # beats the split's latency win now that DVE is near-saturated)
            nc.vector.tensor_scalar_max(z_sb[:], z_ps[:], 0.0)
            return z_sb

        def emit_L4(c, t, g_sb):
            # L4 transposed: lhsT=g (M=128 batch cols), rhs=w4 (N=2) ->
            # out [b, node] with nodes on the PSUM free axis
            if c not in obanks:
                obanks[c] = (
                    ps_o.tile([128, 4 * N], F32, tag="o", name=f"o_{c}"),
                    apool.tile([128, 4 * N], F32, tag="osb", name=f"osb_{c}"))
            o_bank, o_sb = obanks[c]
            w4t = w4_sb[:, t * 2 : (t + 1) * 2]
            for bb in range(4):
                mm(
                    o_bank[:, bb * N + 2 * t : bb * N + 2 * t + 2],
                    g_sb[:, bb * 128 : (bb + 1) * 128],
                    w4t[:],
                    start=True, stop=True)
            if t == npair - 1:
                nc.scalar.activation(o_sb[:], o_bank[:], Copy)
                nc.sync.dma_start(
                    out_d[c * CH : (c + 1) * CH, :].rearrange(
                        "(bb p) n -> p bb n", p=128),
                    o_sb[:].rearrange("p (bb n) -> p bb n", n=N))

        z_cur = emit_L1(*iters[0])
        pend_L4 = None  # (c, t, g_sb) awaiting emission
        for k, (c, t) in enumerate(iters):
            cs = slice(c * CH, (c + 1) * CH)
            xt_c = xt_sb[:, cs]
            w2t = w2_sb[:, t * 128 : (t + 1) * 128]
            w3t = w3_sb[:, t * 128 : (t + 1) * 128]
            w3xt2 = w3x_sb[:, t * 128 : (t + 1) * 128]
            z_sb = z_cur

            # L2: block-diag (K=128, M=64) per node, packed into one PSUM
            r_ps = ps_r.tile([128, CH], F32, tag="r")
            mm(r_ps[0:64, :], w2t[:, 0:64], z_sb[:, 0:CH], start=True, stop=True,
                             tile_position=(0, 0))
            mm(r_ps[64:128, :], w2t[:, 64:128], z_sb[:, CH : 2 * CH], start=True, stop=True,
                             tile_position=(0, 64))
            f_sb = apool.tile([128, CH], BF16, tag="f")
            nc.scalar.activation(f_sb[:], r_ps[:], Relu)

            # L3: x2-terms start the accumulation (diagonal 64x64 quadrants,
            # one concurrent window); the main is a single K=128 M=128 mm on
            # the block-diagonal w3 (off-diag blocks are zero) that stops it.
            h_ps = ps_h.tile([128, CH], F32, tag="h")
            mm(h_ps[0:64, :], w3xt2[0:64, 0:64], xt_c[0:64, :], start=True, stop=False, skip_group_check=True,
               tile_position=(0, 0))
            mm(h_ps[64:128, :], w3xt2[64:128, 64:128], xt_c[64:128, :], start=True, stop=False, skip_group_check=True,
               tile_position=(64, 64))
            mm(h_ps[:, :], w3t[:, :], f_sb[:, :], start=False, stop=True, skip_group_check=True,
               tile_position=(0, 0))
            g_sb = apool.tile([128, CH], BF16, tag="g")
            nc.scalar.activation(g_sb[:], h_ps[:], Relu, bias=b3a_sb[:, t : t + 1])

            if k + 1 < len(iters):
                z_cur = emit_L1(*iters[k + 1])
            # L4(k-1) is emitted AFTER L1(k+1): it depends on g(k-1) (an ACT
            # output), so issuing it earlier head-of-line-blocks the in-order
            # PE queue (~200ns/iter measured)
            if pend_L4 is not None:
                emit_L4(*pend_L4)
            pend_L4 = (c, t, g_sb)
        emit_L4(*pend_L4)

    nc.compile()
    return nc


def _prep_weights(W1a, W1b, W2a, W2b, W3a, b3a, W3b, npair=NPAIR):
    import ml_dtypes
    n = W1a.shape[0]
    mask = (1.0 - np.eye(n, dtype=np.float32))  # [i, n]
    W1am = W1a * mask[:, None, :]
    W2am = W2a * mask[:, None, :]
    w1 = np.zeros((npair, 128, 128), np.float32)
    w2 = np.zeros((npair, 128, 128), np.float32)
    w3 = np.zeros((npair, 128, 128), np.float32)
    w3x = np.zeros((npair, 128, 128), np.float32)
    w4 = np.zeros((npair, 128, 2), np.float32)
    b3ap = np.zeros((128, npair), np.float32)
    for t in range(npair):
        i0, i1 = 2 * t, 2 * t + 1
        w1[t, 0:64, 0:64] = W1am[i0].T
        w1[t, 0:64, 64:128] = W2am[i0].T
        w1[t, 64:128, 0:64] = W1am[i1].T
        w1[t, 64:128, 64:128] = W2am[i1].T
        w2[t, 0:64, 0:32] = W1b[i0].T
        w2[t, 64:128, 32:64] = W2b[i0].T
        w2[t, 0:64, 64:96] = W1b[i1].T
        w2[t, 64:128, 96:128] = W2b[i1].T
        w3[t, 0:64, 0:64] = W3a[i0][:, 0:64].T
        w3[t, 64:128, 64:128] = W3a[i1][:, 0:64].T
        w3x[t, i0, 0:64] = W3a[i0][:, 64 + i0]
        w3x[t, 64 + i0, 0:64] = W3a[i0][:, 64 + i0]
        w3x[t, i1, 64:128] = W3a[i1][:, 64 + i1]
        w3x[t, 64 + i1, 64:128] = W3a[i1][:, 64 + i1]
        w4[t, 0:64, 0] = W3b[i0, 0]
        w4[t, 64:128, 1] = W3b[i1, 0]
        b3ap[0:64, t] = b3a[i0]
        b3ap[64:128, t] = b3a[i1]
    # pack pair-major arrays into the SBUF layout [P, npair*F], bf16
    pk = lambda a: np.ascontiguousarray(
        a.transpose(1, 0, 2).reshape(a.shape[1], -1)).astype(ml_dtypes.bfloat16)
    return pk(w1), pk(w2), pk(w3), pk(w3x), pk(w4), b3ap


def _make_in_maps(inputs):
    """FULL input dict -> per-core in_maps list (host-side pack/shard)."""
    import ml_dtypes
    x = np.asarray(inputs["x"], np.float32)
    w1, w2, w3, w3x, w4, b3ap = _prep_weights(
        np.asarray(inputs["W1a"], np.float32), np.asarray(inputs["W1b"], np.float32),
        np.asarray(inputs["W2a"], np.float32), np.asarray(inputs["W2b"], np.float32),
        np.asarray(inputs["W3a"], np.float32), np.asarray(inputs["b3a"], np.float32),
        np.asarray(inputs["W3b"], np.float32))
    in_maps = []
    for core in range(NCORES):
        xs = x[core * BL : (core + 1) * BL]            # [BL, 64]
        xt = np.ascontiguousarray(
            np.concatenate([xs.T, xs.T], axis=0)).astype(ml_dtypes.bfloat16)
        in_maps.append({"xt": xt, "w1": w1, "w2": w2, "w3": w3,
                        "w3x": w3x, "w4": w4, "b3a": b3ap})
    return in_maps


def kernel(x, W1a, W1b, W2a, W2b, W3a, b3a, W3b, b3b):
    from concourse.bass_utils import run_bass_kernel_spmd

    in_maps = _make_in_maps({"x": x, "W1a": W1a, "W1b": W1b, "W2a": W2a,
                             "W2b": W2b, "W3a": W3a, "b3a": b3a, "W3b": W3b})
    b3b = np.asarray(b3b, np.float32)

    if "nc" not in _cache:
        _cache["nc"] = _build_bass(BL, NPAIR)
    nc = _cache["nc"]

    res = run_bass_kernel_spmd(nc, in_maps, core_ids=list(range(NCORES)))
    out = np.empty((B, N), np.float32)
    for core in range(NCORES):
        opre = res.results[core]["opre"]               # [BL, 64]
        out[core * BL : (core + 1) * BL] = np.maximum(opre + b3b[:, 0][None, :], 0.0)
    return out

